# revision 27
# baseline (speedup 1.0000x reference)
"""BiFormer sparse attention on 8 Trainium2 NeuronCores — gathered top-k.

Problem (hardcoded): B=4, N=2048, C=768, H=12, hd=64, keep=N/2=1024.
    qkv = x @ w_qkv -> q,k,v per (B,H)
    top-1024 tokens per (B,H) by ||q|| -> gather k,v
    out = softmax(clip(q @ k_sel^T * hd^-0.5, +-50)) @ v_sel
    y = clip(out @ w_proj + b_proj, +-10)

Sharding: 8 cores = 4 batches x 2 head-groups (6 heads each). Weights are
column/row-split per head-group; the two cores of a batch produce partial
projection outputs that the host sums (+bias, clip).

Device algorithm (per core). Gathers the top-1024 keys/values per head,
halving the S/exp/PV work vs masked attention over all 2048 keys:
  1. q^T per head (f32r matmuls), squares -> f32 scores[token, head].
     k+v packed per head into internal-DRAM rows kvnat[tok, h, 0:256] =
     [k 2-head 128-ch window | v 64 ch + ones col + pad], bf16 — one
     512-byte gather element per (head, token).
  2. Per-head top-1024 threshold by bisection with the state replicated
     [128, 6] (count via ones128 matmul -> every partition holds the
     count, so no PE broadcast per iteration). Overlaps the k/v matmuls.
  3. Index build: masked ids m?token_id:-1 -> PE identity-matmul
     reshuffle into the 16-partition-wrapped layout -> gpsimd
     sparse_gather compacts each head's kept ids in order -> cast int16,
     replicate to 128 partitions (DGE index format).
  4. One dma_gather per head -> kvsel[128, 8, 256]; PE-mode transposes
     give the channel-major k_sel^T blocks for the S matmul lhsT.
  5. Attention per (query-chunk, head): S^T over 8 key blocks, exp on
     ACT (no mask needed), PV accumulates [65, 512] (row 64 = denom).
     Normalize via DVE reciprocal + PE outer-product broadcast.
     Project with row-split w_proj per chunk.
Bulk DMAs (x in, kvnat out, y out) issue from the otherwise-idle Sync
engine so the GpSimd queue is free for the serial gather/sparse preps.
"""
import os
import sys

sys.path.insert(0, "/opt/trn_rl_repo")

import numpy as np

import concourse.bass as bass
import concourse.mybir as mybir
from concourse import bacc
from concourse.tile import TileContext
from concourse.bass_utils import run_bass_kernel_spmd

B, N, C, H, HD = 4, 2048, 768, 12, 64
HPC = 6                  # heads per core
QD = HPC * HD            # 384 q/k/v channels per core
KEEP = N // 2            # 1024
NB = N // 128            # 16 token blocks
QC = N // 512            # 4 query chunks
CB = C // 128            # 6 contraction blocks
KG = KEEP // 128         # 8 gathered key blocks
SCALE = HD ** -0.5       # 0.125
BISECT_HI = 512.0        # scores are chi2(64)-like, max ~150 << 512
BISECT_ITERS = 9
F32 = mybir.dt.float32
F32R = mybir.dt.float32r
BF16 = mybir.dt.bfloat16
F16 = mybir.dt.float16
I16 = mybir.dt.int16

_CACHE = {}
TRACE = False       # set True (e.g. from test.py) to capture an NTFF profile
LAST = {}           # exec_time_ns / profile info from the most recent run
KPHASE = int(os.environ.get("KPHASE", "5"))  # debug: truncate kernel after phase


def _build():
    nc = bacc.Bacc(None, target_bir_lowering=False)
    xT_d = nc.declare_dram_parameter("xT", [C, N], F32, isOutput=False)
    wq_d = nc.declare_dram_parameter("wq", [C, QD], F32, isOutput=False)
    wk_d = nc.declare_dram_parameter("wk", [C, QD], F32, isOutput=False)
    wv_d = nc.declare_dram_parameter("wv", [C, QD], F32, isOutput=False)
    wp_d = nc.declare_dram_parameter("wp", [QD, C], F32, isOutput=False)
    sel_d = nc.declare_dram_parameter("selmask", [QD, HPC], F32, isOutput=False)
    iota_d = nc.declare_dram_parameter("iotap1", [128, NB], F16, isOutput=False)
    id_d = nc.declare_dram_parameter("ident", [128, 128], F16, isOutput=False)
    idb_d = nc.declare_dram_parameter("identf32", [128, 128], F32, isOutput=False)
    offs_d = nc.declare_dram_parameter("probeoffs", [128, 8], F32, isOutput=False)
    rep_d = nc.declare_dram_parameter("rep16", [16, 128], F32, isOutput=False)
    y_d = nc.declare_dram_parameter("y", [N, C], F32, isOutput=True)
    thr_d = nc.declare_dram_parameter("dbg_thr", [1, HPC], F32, isOutput=True)
    idx_d = nc.declare_dram_parameter("dbg_idx", [128, HPC * 64], F32, isOutput=True)

    with TileContext(nc) as tc:
        with (
            tc.tile_pool(name="wts", bufs=1) as wts,
            tc.tile_pool(name="xp", bufs=1) as xp,
            tc.tile_pool(name="qt", bufs=1) as qtp,
            tc.tile_pool(name="sq", bufs=1) as sqp,
            tc.tile_pool(name="stg", bufs=4) as stg,
            tc.tile_pool(name="bis", bufs=2) as bis,
            tc.tile_pool(name="idx", bufs=1) as idxp,
            tc.tile_pool(name="ksel", bufs=1) as ksp,
            tc.tile_pool(name="pt", bufs=12) as ptp,
            tc.tile_pool(name="outt", bufs=1) as otp,
            tc.tile_pool(name="y", bufs=2) as yp,
            tc.tile_pool(name="small", bufs=1) as sml,
            tc.tile_pool(name="dram", bufs=1, space="DRAM") as drp,
            tc.tile_pool(name="mm", bufs=4, space="PSUM") as pmm,
            tc.tile_pool(name="acc", bufs=3, space="PSUM") as pacc,
            tc.tile_pool(name="bmm", bufs=1, space="PSUM") as pbis,
        ):
            # ---- x (resident, f32r via cast-DMA) ----
            xt = [xp.tile([128, N], F32R, tag=f"x{kb}", name=f"x{kb}")
                  for kb in range(CB)]
            _xeng = [nc.sync, nc.scalar]
            for nb in range(QC):
                for kb in range(CB):
                    _xeng[(nb * CB + kb) % 2].dma_start(
                        out=xt[kb][:, nb * 512:(nb + 1) * 512],
                        in_=xT_d[kb * 128:(kb + 1) * 128,
                                 nb * 512:(nb + 1) * 512].bitcast(F32R))

            # ---- weights / constants ----
            def load32(dram, cols, n, tag):
                ts = []
                for i in range(n):
                    t = wts.tile([128, cols], F32R, tag=f"{tag}{i}", name=f"{tag}{i}")
                    nc.gpsimd.dma_start(out=t, in_=dram[i * 128:(i + 1) * 128, :])
                    ts.append(t)
                return ts

            wq = load32(wq_d, QD, CB, "wq")
            wk = load32(wk_d, QD, CB, "wk")
            wv = load32(wv_d, QD, CB, "wv")
            wp = []
            for i in range(3):
                t = wts.tile([128, C], BF16, tag=f"wp{i}", name=f"wp{i}")
                nc.gpsimd.dma_start(out=t, in_=wp_d[i * 128:(i + 1) * 128, :])
                wp.append(t)
            selm = []
            for i in range(3):
                st = sml.tile([128, HPC], F32, tag=f"selm{i}", name=f"selm{i}")
                nc.gpsimd.dma_start(out=st, in_=sel_d[i * 128:(i + 1) * 128, :])
                selm.append(st)
            iota_sb = sml.tile([128, NB], F16, tag="iota")
            nc.gpsimd.dma_start(out=iota_sb, in_=iota_d[:, :])
            id_sb = sml.tile([128, 128], F16, tag="ident")
            nc.gpsimd.dma_start(out=id_sb, in_=id_d[:, :])
            id_bf = sml.tile([128, 128], BF16, tag="identbf")
            nc.gpsimd.dma_start(out=id_bf, in_=idb_d[:, :])
            rep16 = sml.tile([16, 128], F32, tag="rep16")
            nc.gpsimd.dma_start(out=rep16, in_=rep_d[:, :])
            offs_sb = sml.tile([128, 8], F32, tag="offs")
            nc.gpsimd.dma_start(out=offs_sb, in_=offs_d[:, :])
            ones128 = sml.tile([128, 128], BF16, tag="ones128")
            nc.vector.memset(ones128, 1.0)


            # ---- phase 1a: q^T per head + squares + scores ----
            qT = [qtp.tile([64, N], BF16, tag=f"qT{h}", name=f"qT{h}")
                  for h in range(HPC)]
            scores = bis.tile([128, HPC, NB], F32, tag="scores", bufs=1)
            for nb in range(QC):
                nsl = slice(nb * 512, (nb + 1) * 512)
                sq_c = [sqp.tile([128, 512], F32, tag=f"sq{m}", name=f"sq{m}", bufs=1)
                        for m in range(3)]
                for mb in range(3):
                    ps = pmm.tile([128, 512], F32, tag="mm", name="psq")
                    for kb in range(CB):
                        nc.tensor.matmul(
                            ps, wq[kb][:, mb * 128:(mb + 1) * 128],
                            xt[kb][:, nsl],
                            start=(kb == 0), stop=(kb == CB - 1))
                    nc.scalar.activation(
                        sq_c[mb], ps, mybir.ActivationFunctionType.Square)
                    nc.scalar.activation(qT[2 * mb][:, nsl], ps[0:64, :],
                                         mybir.ActivationFunctionType.Copy)
                    nc.scalar.activation(qT[2 * mb + 1][:, nsl], ps[64:128, :],
                                         mybir.ActivationFunctionType.Copy)
                for j in range(4):
                    tb = nb * 4 + j
                    pss = pmm.tile([128, HPC], F32, tag="mm", name="pssc")
                    for m in range(3):
                        nc.tensor.matmul(
                            pss, sq_c[m][:, j * 128:(j + 1) * 128], selm[m],
                            start=(m == 0), stop=(m == 2))
                    nc.vector.tensor_copy(scores[:, :, tb], pss)

            # ---- phase 2: multi-probe bisection (8 probes/iter, 9 iters) ----
            # state replicated [128, HPC] so no cross-partition traffic;
            # probes p_j = c - w + 2w(j+1)/9 divide (c-w, c+w) into 9 parts
            NP = 8
            thr128 = bis.tile([128, HPC], F32, tag="thr")
            lo128 = bis.tile([128, HPC], F32, tag="lo")
            if KPHASE >= 2:
                nc.vector.memset(thr128, BISECT_HI / 2)
                nc.vector.memset(lo128, 0.0)
                w = BISECT_HI / 2
                for it in range(BISECT_ITERS):
                    probes = bis.tile([128, HPC, NP], F32, tag="probes",
                                      name="probes")
                    nc.vector.scalar_tensor_tensor(
                        out=probes,
                        in0=offs_sb.unsqueeze(1).to_broadcast([128, HPC, NP]),
                        scalar=w,
                        in1=thr128.unsqueeze(-1).to_broadcast([128, HPC, NP]),
                        op0=mybir.AluOpType.mult, op1=mybir.AluOpType.add)
                    cmp = bis.tile([128, HPC, NP, NB], BF16, tag="cmp",
                                   name="cmp")
                    nc.vector.tensor_tensor(
                        cmp,
                        scores.unsqueeze(2).to_broadcast([128, HPC, NP, NB]),
                        probes.unsqueeze(-1).to_broadcast([128, HPC, NP, NB]),
                        op=mybir.AluOpType.is_ge)
                    red1 = bis.tile([128, HPC, NP], BF16, tag="red1",
                                    name="red1")
                    with nc.allow_low_precision(reason="block counts <= 16 are bf16-exact"):
                        nc.vector.tensor_reduce(
                            red1, cmp, axis=mybir.AxisListType.X,
                            op=mybir.AluOpType.add)
                    pc = pbis.tile([128, HPC * NP], F32, tag="bmm",
                                   name="pscnt")
                    nc.tensor.matmul(
                        pc, ones128, red1.rearrange("p a b -> p (a b)"),
                        start=True, stop=True)
                    sel = bis.tile([128, HPC, NP], F32, tag="sel", name="sel")
                    nc.vector.tensor_scalar(
                        sel, pc.rearrange("p (a b) -> p a b", a=HPC),
                        float(KEEP), None, op0=mybir.AluOpType.is_ge)
                    sfn = bis.tile([128, HPC], F32, tag="sfn", name="sfn")
                    nc.vector.tensor_reduce(
                        sfn, sel, axis=mybir.AxisListType.X,
                        op=mybir.AluOpType.add)
                    # cand = (c - w) + s*(2w/9) = p_{s-1};  c' = cand + w/9
                    tmp = bis.tile([128, HPC], F32, tag="tmp", name="tmp")
                    nc.vector.tensor_scalar(
                        tmp, thr128, w, None, op0=mybir.AluOpType.subtract)
                    cand = bis.tile([128, HPC], F32, tag="cand", name="cand")
                    nc.vector.scalar_tensor_tensor(
                        out=cand, in0=sfn, scalar=2.0 * w / (NP + 1), in1=tmp,
                        op0=mybir.AluOpType.mult, op1=mybir.AluOpType.add)
                    selu = bis.tile([128, HPC], mybir.dt.uint32, tag="selu",
                                    name="selu")
                    nc.vector.tensor_scalar(
                        selu, sfn, 1.0, None, op0=mybir.AluOpType.is_ge)
                    # lo keeps the highest probe known to satisfy count>=KEEP
                    nc.vector.select(lo128, selu, cand, lo128)
                    nc.vector.tensor_scalar(
                        thr128, cand, -w / (NP + 1), None,
                        op0=mybir.AluOpType.subtract)
                    w /= (NP + 1)
                nc.gpsimd.dma_start(out=thr_d[:, :], in_=lo128[0:1, :])

            # ---- phase 1b/1c: k+v natural, packed per head -> DRAM ----
            # kvnat[tok, h, 0:128]   = k channels [64h, 64h+128) (2-head win)
            # kvnat[tok, h, 128:193] = v channels of head h + ones col
            kvnat = drp.tile([N, HPC, 256], BF16)
            if KPHASE >= 1:
                for tb in range(NB):
                    tsl = slice(tb * 128, (tb + 1) * 128)
                    ps = pmm.tile([128, QD], F32, tag="mm", name="psk")
                    for kb in range(CB):
                        nc.tensor.matmul(
                            ps, xt[kb][:, tsl], wk[kb],
                            start=(kb == 0), stop=(kb == CB - 1))
                    ksb = stg.tile([128, HPC, 128], BF16, tag="ksb", name="ksb")
                    nc.scalar.activation(
                        ksb[:, :, 0:64],
                        ps.rearrange("p (h d) -> p h d", h=HPC),
                        mybir.ActivationFunctionType.Copy)
                    nc.scalar.activation(
                        ksb[:, 0:HPC - 1, 64:128],
                        ps[:, 64:QD].rearrange("p (h d) -> p h d", h=HPC - 1),
                        mybir.ActivationFunctionType.Copy)
                    nc.sync.dma_start(out=kvnat[tsl, :, 0:128], in_=ksb)
                for tb in range(NB):
                    tsl = slice(tb * 128, (tb + 1) * 128)
                    ps = pmm.tile([128, QD], F32, tag="mm", name="psv")
                    for kb in range(CB):
                        nc.tensor.matmul(
                            ps, xt[kb][:, tsl], wv[kb],
                            start=(kb == 0), stop=(kb == CB - 1))
                    vsb = stg.tile([128, HPC, 65], BF16, tag="vsb", name="vsb")
                    nc.scalar.activation(
                        vsb[:, :, 0:64], ps.rearrange("p (h d) -> p h d", h=HPC),
                        mybir.ActivationFunctionType.Copy)
                    nc.vector.memset(vsb[:, :, 64:65], 1.0)
                    nc.sync.dma_start(out=kvnat[tsl, :, 128:193], in_=vsb)

            if KPHASE >= 3:
                # ---- phase 3: compacted per-head index lists ----
                cmpm = bis.tile([128, HPC, NB], F16, tag="cmpm", bufs=1)
                nc.vector.tensor_tensor(
                    cmpm, scores,
                    lo128.unsqueeze(-1).to_broadcast([128, HPC, NB]),
                    op=mybir.AluOpType.is_ge)
                maskedm = bis.tile([128, HPC, NB], F16, tag="maskedm", bufs=1)
                nc.vector.tensor_tensor(
                    maskedm, cmpm,
                    iota_sb.unsqueeze(1).to_broadcast([128, HPC, NB]),
                    op=mybir.AluOpType.mult)
                nc.vector.tensor_scalar(
                    maskedm, maskedm, 1.0, None, op0=mybir.AluOpType.subtract)
                # reshuffle [128, h, c] -> wrapped[pl, h, c, ph] via identity
                # matmuls (token 128c+16ph+pl lands at wrapped col 128h+8c+ph,
                # i.e. logical position 2048h+t of the 16-wrapped stream)
                wrapped = idxp.tile([16, HPC, NB, 8], F16, tag="wrapped")
                for ph in range(8):
                    pw = pbis.tile([16, HPC * NB], F32, tag="bmm", name="pw")
                    nc.tensor.matmul(
                        pw, id_sb[:, 16 * ph:16 * ph + 16],
                        maskedm.rearrange("p a b -> p (a b)"),
                        start=True, stop=True)
                    nc.vector.tensor_copy(
                        wrapped[:, :, :, ph],
                        pw.rearrange("p (a b) -> p a b", a=HPC))
                # per-head compaction (robust to >KEEP kept via 4 pad cols),
                # then replicate the 16-partition wrap to all 128 partitions
                # via a 0/1 PE matmul (no gpsimd round trip)
                idxr = idxp.tile([128, HPC * 64], I16, tag="idxr")
                pidx = pbis.tile([128, HPC, 64], F32, tag="bmm", name="pidx")
                for h in range(HPC):
                    idxf = idxp.tile([16, 68], F32, tag=f"idxf{h}", name=f"idxf{h}")
                    nfnd = idxp.tile([1, 1], mybir.dt.uint32, tag=f"nf{h}",
                                     name=f"nf{h}")
                    nc.gpsimd.sparse_gather(
                        idxf, wrapped[:, h].rearrange("p a b -> p (a b)"),
                        num_found=nfnd)
                    nc.tensor.matmul(pidx[:, h, :], rep16, idxf[:, 0:64],
                                     start=True, stop=True)
                nc.vector.tensor_copy(idxr.rearrange("p (a b) -> p a b", a=HPC),
                                      pidx)

            if KPHASE == 3:
                dbg = idxp.tile([128, HPC * 64], F32, tag="dbgidx")
                nc.vector.tensor_copy(dbg, idxr)
                nc.gpsimd.dma_start(out=idx_d[:, :], in_=dbg)

            if KPHASE >= 4:
                # ---- phase 4: one gather per head + PE-transpose of k ----
                kT_sel = [ksp.tile([64, KEEP], BF16, tag=f"ks{h}", name=f"ks{h}")
                          for h in range(HPC)]
                kvsel = [ksp.tile([128, KG, 256], BF16, tag=f"kv{h}", name=f"kv{h}")
                         for h in range(HPC)]
                def gather_dma(h):
                    nc.gpsimd.dma_gather(
                        kvsel[h][:, :, :], kvnat[:, h, 0:256],
                        idxr[:, 64 * h:64 * h + 64], KEEP, KEEP, 256,
                        elem_step=HPC * 256, transpose=False)

                def transpose_head(h):
                    # PE-transpose each 128-key block to channel-major
                    for gk in range(KG):
                        ptr = pmm.tile([64, 128], BF16, tag="mm", name="ptr")
                        nc.tensor.transpose(ptr, kvsel[h][:, gk, 0:64], id_bf)
                        nc.vector.tensor_copy(
                            kT_sel[h][:, gk * 128:(gk + 1) * 128], ptr)

                def gather_head(h):
                    gather_dma(h)
                    transpose_head(h)

            if KPHASE == 4:
                for h in range(HPC):
                    gather_head(h)
                # dump gathered head-0 k^T and v for verification
                dbgk = idxp.tile([64, 192], F32, tag="dbgk")
                nc.vector.tensor_copy(dbgk, kT_sel[0][:, 0:192])
                nc.gpsimd.dma_start(out=idx_d[0:64, 0:192], in_=dbgk)
                dbgv = idxp.tile([128, 192], F32, tag="dbgv")
                nc.vector.tensor_copy(
                    dbgv[:, 0:128], kvsel[0][:, 0, 128:256])
                nc.vector.tensor_copy(
                    dbgv[:, 128:192], kvsel[0][:, 1, 128:192])
                nc.gpsimd.dma_start(out=idx_d[:, 192:384], in_=dbgv)

            if KPHASE >= 5:
                # ---- phase 5: attention + projection ----
                outT = [otp.tile([128, N], BF16, tag=f"outT{i}", name=f"outT{i}")
                        for i in range(3)]
                ones64 = sml.tile([1, 64], F32, tag="ones64")
                nc.vector.memset(ones64, 1.0)
                def emit_proj_qb(qb):
                    if True:
                        ps1 = pmm.tile([128, 512], F32, tag="mm", name="psy1")
                        ps2 = pmm.tile([128, 256], F32, tag="mm", name="psy2")
                        for i in range(3):
                            lhsT = outT[i][:, qb * 128:(qb + 1) * 128]
                            nc.tensor.matmul(ps1, lhsT, wp[i][:, 0:512],
                                             start=(i == 0), stop=(i == 2))
                            nc.tensor.matmul(ps2, lhsT, wp[i][:, 512:768],
                                             start=(i == 0), stop=(i == 2))
                        yt = yp.tile([128, C], F32, tag="y", name="yt")
                        nc.vector.tensor_copy(yt[:, 0:512], ps1)
                        nc.vector.tensor_copy(yt[:, 512:768], ps2)
                        nc.sync.dma_start(
                            out=y_d[qb * 128:(qb + 1) * 128, :], in_=yt)

                def attend(qc, hs, proj_qc=None):
                    qsl = slice(qc * 512, (qc + 1) * 512)
                    if True:
                        po = {h: pacc.tile([HD + 1, 512], F32, tag="acc",
                                           name=f"po{h % 2}") for h in hs}
                        # heads interleaved, PV lagging by 2 blocks: PE always
                        # has an independent matmul while ACT works
                        pipe = []
                        for gk in range(KG):
                            cur = []
                            for h in hs:
                                ps = pmm.tile([128, 512], F32, tag="mm",
                                              name="psS")
                                nc.tensor.matmul(
                                    ps, kT_sel[h][:, gk * 128:(gk + 1) * 128],
                                    qT[h][:, qsl], start=True, stop=True)
                                pt = ptp.tile([128, 512], BF16, tag="pt",
                                              name="pt")
                                nc.scalar.activation(
                                    pt, ps, mybir.ActivationFunctionType.Exp,
                                    scale=SCALE)
                                cur.append((h, pt))
                            pipe.append((gk, cur))
                            if proj_qc is not None and gk % 2 == 1:
                                emit_proj_qb(4 * proj_qc + gk // 2)
                            if len(pipe) > 2:
                                pg, pts = pipe.pop(0)
                                for h, ppt in pts:
                                    nc.tensor.matmul(
                                        po[h], kvsel[h][:, pg, 128:193], ppt,
                                        start=(pg == 0), stop=False)
                        for pg, pts in pipe:
                            for h, ppt in pts:
                                nc.tensor.matmul(
                                    po[h], kvsel[h][:, pg, 128:193], ppt,
                                    start=(pg == 0), stop=(pg == KG - 1))
                        # normalize rows 0..63 by 1/row64
                        for h in hs:
                            den = sml.tile([1, 512], F32, tag="den", name="den",
                                           bufs=2)
                            nc.vector.tensor_copy(den, po[h][HD:HD + 1, :])
                            recip = sml.tile([1, 512], F32, tag="recip",
                                             name="recip", bufs=2)
                            nc.vector.reciprocal_approx_fast(out=recip, in_=den)
                            rps = sml.tile([HD, 512], F32, tag="reps",
                                           name="reps", bufs=2)
                            nc.gpsimd.partition_broadcast(rps, recip)
                            nc.vector.tensor_mul(
                                outT[h // 2][64 * (h % 2):64 * (h % 2) + 64, qsl],
                                po[h][0:HD, :], rps)

                # Emission order fills the serial gather train: each head's
                # gather is chased by its qc0 attention, and the early-head
                # trios of later chunks slot between the remaining gathers.
                for h in range(3):
                    gather_head(h)
                    attend(0, (h,))
                for i, qc in enumerate((1, 2, 3)):
                    gather_dma(3 + i)
                    attend(qc, (0, 1, 2))
                    transpose_head(3 + i)
                    attend(0, (3 + i,))
                for qc in range(1, QC):
                    attend(qc, (3, 4, 5), proj_qc=qc - 1)
                for qb in range(4 * (QC - 1), 4 * QC):
                    emit_proj_qb(qb)


    nc.compile()
    return nc


def _get_nc():
    if "nc" not in _CACHE:
        _CACHE["nc"] = _build()
    return _CACHE["nc"]


def kernel(x, w_qkv, w_proj, b_proj):
    x = np.asarray(x, dtype=np.float32)
    w_qkv = np.asarray(w_qkv, dtype=np.float32)
    w_proj = np.asarray(w_proj, dtype=np.float32)
    b_proj = np.asarray(b_proj, dtype=np.float32)

    selmask = np.zeros((QD, HPC), dtype=np.float32)
    for h in range(HPC):
        selmask[h * HD:(h + 1) * HD, h] = 1.0
    iotap1 = (np.arange(128, dtype=np.float32)[:, None]
              + 128.0 * np.arange(NB, dtype=np.float32)[None, :] + 1.0
              ).astype(np.float16)
    ident = np.eye(128, dtype=np.float16)

    in_maps = []
    for core in range(8):
        b, g = core // 2, core % 2
        cols = slice(g * QD, (g + 1) * QD)
        in_maps.append({
            "xT": np.ascontiguousarray(x[b].T),
            "wq": np.ascontiguousarray(w_qkv[:, 0:C][:, cols]),
            "wk": np.ascontiguousarray(w_qkv[:, C:2 * C][:, cols]),
            "wv": np.ascontiguousarray(w_qkv[:, 2 * C:3 * C][:, cols]),
            "wp": np.ascontiguousarray(w_proj[cols, :]),
            "selmask": selmask,
            "iotap1": iotap1,
            "ident": ident,
            "identf32": np.eye(128, dtype=np.float32),
            "probeoffs": np.tile((2.0 * (np.arange(8) + 1) / 9.0 - 1.0
                                  ).astype(np.float32), (128, 1)),
            "rep16": np.tile(np.eye(16, dtype=np.float32), (1, 8)),
        })

    nc = _get_nc()
    r = run_bass_kernel_spmd(nc, in_maps, list(range(8)), trace=TRACE)
    LAST["exec_time_ns"] = r.exec_time_ns
    LAST["mean_exec_time_ns"] = r.mean_exec_time_ns
    LAST["results"] = r.results
    LAST["insts"] = r.instructions_and_trace
    y = np.empty((B, N, C), dtype=np.float32)
    for b in range(B):
        y[b] = r.results[2 * b]["y"] + r.results[2 * b + 1]["y"]
    y = np.clip(y + b_proj, -10.0, 10.0)
    return y
